# revision 1
# baseline (speedup 1.0000x reference)
"""Trainium2 Bass kernel for nn_DynConv2d (DGCNN EdgeConv layer).

Reference computation (B=2, C=64, N=8192, K=16, C_out=64):
  f = x[:,:,:,0]
  nn_idx = top-16 nearest neighbors by squared L2 over point features
  feat = concat([x_i, x_j - x_i])          # (B, 2C, N, K)
  y = W @ feat                             # 1x1 conv
  y = BatchNorm2d(y)  (training stats over (B,N,K))
  y = LeakyReLU(0.2)(y)
  out = max over K                         # (B, C_out, N)

Key algebraic restructuring used here:
  * W @ [x_i; x_j - x_i] = (W1-W2) @ x_i + W2 @ x_j = u[:,i] + v[:,j]
    with u = (W1-W2) @ f, v = W2 @ f   (two tiny 64x64 matmuls)
  * BN (gamma=1>0) + LeakyReLU is monotone increasing per channel, so
    max over K commutes:  out = lrelu(a * max_k(u+v_j) + b).
    The BN batch stats need sum(y) and sum(y^2) over (B,N,K), which the
    kernel accumulates on-device; the final per-channel affine + lrelu is
    applied on host (trivial elementwise pass).
  * KNN scores: argtop16_m of (q . m - |m|^2/2), computed via a single
    65-contraction matmul using an augmented row of ones / -|m|^2/2.

Sharding: 8 cores; core c handles batch c//4, query block c%4 (2048 queries),
against all 8192 keys of its batch.

Top-16 per query row (8192 scores, fp32 exact):
  L1: nc.vector.max (top-8) per 512-chunk -> 128 candidates
      nc.vector.max_index per chunk -> chunk-local indices -> +chunk base
  L2: max8 + match_replace + max8 -> top-16 values
      max_index on candidate array -> candidate positions
  map: two gpsimd local_scatter ops route candidate global indices to the
      16 winner slots (per-partition scatter).
  (Exactness requires no query has >8 of its top-16 in one 512-chunk;
   astronomically likely and verified against the reference offline.)

Neighbor features are fetched with one indirect DMA (row gather) per query
tile from a DRAM copy of v^T, then max/sum-reduced along K on-chip.
"""

import os
import sys

import numpy as np

sys.path.insert(0, "/opt/trn_rl_repo")

import concourse.bacc as bacc
import concourse.bass as bass
import concourse.mybir as mybir
import concourse.tile as tile
from concourse.masks import make_identity

F32 = mybir.dt.float32
U16 = mybir.dt.uint16
I16 = mybir.dt.int16
U32 = mybir.dt.uint32

BN_EPS = 1e-5
LRELU_SLOPE = 0.2


def default_cfg():
    return dict(C=64, NK=8192, NQ=2048, K=16, KT=512, CH=512)


def emit(tc, ins, outs, cfg, uniq=""):
    """Emit the per-core program.

    ins:  f (C, NK), fq (C, NQ), w2t (C, C), wat (C, C)   [DRAM APs]
    outs: out_m (C, NQ)  max-over-K of u+v (pre-BN),
          out_s (128, 512) rows {0,32,64,96} = psum stats
          (0: sum A cols 0:512, 32: sum A cols 512:1024,
           64: sum A^2 cols 0:512, 96: sum A^2 cols 512:1024)
    """
    nc = tc.nc
    C = cfg["C"]
    NK = cfg["NK"]
    NQ = cfg["NQ"]
    K = cfg["K"]
    KT = cfg["KT"]          # key tile (matmul free dim)
    CH = cfg["CH"]          # L1 top-k chunk size
    NKT = NK // KT
    NQT = NQ // 128
    NCH = NK // CH
    CAND = 8 * NCH          # candidates per query row
    NPC = NK // 128         # 128-point chunks for v^T
    assert CAND <= 32767 and NK <= 16384

    f, fq, w2t, wat = ins["f"], ins["fq"], ins["w2t"], ins["wat"]
    out_m, out_s = outs["out_m"], outs["out_s"]

    from contextlib import ExitStack
    ctx = ExitStack()
    dram_pool = ctx.enter_context(tc.tile_pool(name="dram" + uniq, bufs=1,
                                               space="DRAM"))
    vt_tile = dram_pool.tile([NK, C], F32, tag="vt", name="vt_t" + uniq)
    vt_dram = vt_tile[:]
    cpool = ctx.enter_context(tc.tile_pool(name="consts" + uniq, bufs=1))
    big = ctx.enter_context(tc.tile_pool(name="big" + uniq, bufs=1))
    spool = ctx.enter_context(tc.tile_pool(name="scores" + uniq, bufs=2))
    tk = ctx.enter_context(tc.tile_pool(name="topk" + uniq, bufs=3))
    gpool = ctx.enter_context(tc.tile_pool(name="gather" + uniq, bufs=3))
    ps_score = ctx.enter_context(tc.tile_pool(name="ps_score" + uniq, bufs=4, space="PSUM"))
    ps_stat = ctx.enter_context(tc.tile_pool(name="ps_stat" + uniq, bufs=1, space="PSUM"))
    ps_misc = ctx.enter_context(tc.tile_pool(name="ps_misc" + uniq, bufs=3, space="PSUM"))

    # ---- constants ----
    identity = cpool.tile([128, 128], F32, tag="ident")
    make_identity(nc, identity[:])
    ones_col = cpool.tile([128, 1], F32, tag="ones")
    nc.vector.memset(ones_col[:], 1.0)
    # chunk base for candidate j (0..CAND-1): (j // 8) * CH
    cbase = cpool.tile([128, CAND], U16, tag="cbase")
    nc.gpsimd.iota(cbase[:], pattern=[[CH, NCH], [0, 8]], base=0,
                   channel_multiplier=0)
    # 1..16 for local_scatter slot marking
    iota16 = cpool.tile([128, K], I16, tag="iota16")
    nc.gpsimd.iota(iota16[:], pattern=[[1, K]], base=1, channel_multiplier=0)
    w2t_sb = cpool.tile([C, C], F32, tag="w2t")
    nc.sync.dma_start(out=w2t_sb[:], in_=w2t)
    wat_sb = cpool.tile([C, C], F32, tag="wat")
    nc.sync.dma_start(out=wat_sb[:], in_=wat)

    # ---- load + augment ----
    keys_aug = big.tile([C + 1, NK], F32, tag="keys_aug")
    nc.sync.dma_start(out=keys_aug[0:C, :], in_=f)
    q_aug = big.tile([C + 1, NQ], F32, tag="q_aug")
    nc.sync.dma_start(out=q_aug[0:C, :], in_=fq)
    nc.vector.memset(q_aug[C:C + 1, :], 1.0)

    # f^2 -> column sums -> -|m|^2/2 into keys_aug row C
    f2 = spool.tile([C, NK], F32, tag="S")  # reuse a score slot (prologue only)
    nc.scalar.square(f2[0:C, :], keys_aug[0:C, :])
    for kt in range(NKT):
        ps_sq = ps_misc.tile([128, KT], F32, tag="misc")
        nc.tensor.matmul(ps_sq[64:65, :], lhsT=ones_col[0:C, :],
                         rhs=f2[0:C, bass.ts(kt, KT)], start=True, stop=True,
                         tile_position=(0, 64))
        nc.scalar.mul(keys_aug[C:C + 1, bass.ts(kt, KT)], ps_sq[64:65, :], -0.5)

    # ---- v^T to DRAM (v = W2 @ f, stored point-major), u^T kept in SBUF ----
    vt_big = big.tile([128, NPC * C], F32, tag="vt_big")
    for pc in range(NPC):
        ps_vt = ps_misc.tile([128, 128], F32, tag="misc")
        nc.tensor.matmul(ps_vt[:, 0:C], lhsT=keys_aug[0:C, bass.ts(pc, 128)],
                         rhs=w2t_sb[:], start=True, stop=True)
        nc.scalar.copy(vt_big[:, bass.ts(pc, C)], ps_vt[:, 0:C])
    nc.sync.dma_start(
        out=vt_dram.rearrange("(pc p) o -> p pc o", p=128),
        in_=vt_big[:].rearrange("p (pc o) -> p pc o", pc=NPC),
    )

    ut_sb = big.tile([128, NQT * C], F32, tag="ut")
    for qt in range(NQT):
        ps_ut = ps_misc.tile([128, 128], F32, tag="misc")
        nc.tensor.matmul(ps_ut[:, 0:C], lhsT=q_aug[0:C, bass.ts(qt, 128)],
                         rhs=wat_sb[:], start=True, stop=True)
        nc.scalar.copy(ut_sb[:, bass.ts(qt, C)], ps_ut[:, 0:C])

    outm_sb = big.tile([C, NQ], F32, tag="outm")
    stats_ps = None
    if not cfg.get("no_gather"):
        stats_ps = ps_stat.tile([128, KT], F32, tag="stat")

    # epilogue for tile tq (software-pipelined: called 2 tiles behind)
    def tail(tq):
        G = G_tiles[tq]
        ut_qt = ut_sb[:, bass.ts(tq, C)]
        A = gpool.tile([128, K * C], F32, tag="A", name=f"A_t{uniq}_{tq}")
        nc.gpsimd.tensor_tensor(
            out=A[:].rearrange("p (k o) -> p k o", k=K),
            in0=G[:].rearrange("p (k o) -> p k o", k=K),
            in1=ut_qt.unsqueeze(1).broadcast_to([128, K, C]),
            op=mybir.AluOpType.add)
        O = tk.tile([128, C], F32, tag="O", name=f"O_t{uniq}_{tq}")
        nc.vector.tensor_reduce(
            out=O[:], in_=A[:].rearrange("p (k o) -> p o k", k=K),
            axis=mybir.AxisListType.X, op=mybir.AluOpType.max)
        A2 = gpool.tile([128, K * C], F32, tag="A2", name=f"A2_t{uniq}_{tq}")
        nc.scalar.square(A2[:], A[:])
        st, sp = (tq == 0), (tq == NQT - 1)
        H = K * C // 2
        nc.tensor.matmul(stats_ps[0:1, 0:H], lhsT=ones_col[:], rhs=A[:, 0:H],
                         start=st, stop=sp, tile_position=(0, 0))
        nc.tensor.matmul(stats_ps[32:33, 0:H], lhsT=ones_col[:], rhs=A[:, H:],
                         start=st, stop=sp, tile_position=(0, 32))
        nc.tensor.matmul(stats_ps[64:65, 0:H], lhsT=ones_col[:], rhs=A2[:, 0:H],
                         start=st, stop=sp, tile_position=(0, 64))
        nc.tensor.matmul(stats_ps[96:97, 0:H], lhsT=ones_col[:], rhs=A2[:, H:],
                         start=st, stop=sp, tile_position=(0, 96))
        ps_tr = ps_misc.tile([128, 128], F32, tag="misc", name=f"ps_tr_t{uniq}_{tq}")
        nc.tensor.transpose(ps_tr[0:C, :], O[:], identity[:])
        nc.scalar.copy(outm_sb[:, bass.ts(tq, 128)], ps_tr[0:C, :])

    # ---- main loop over query tiles ----
    G_tiles = {}
    for qt in range(NQT):
        lhsT = q_aug[:, bass.ts(qt, 128)]  # (C+1, 128)

        S = spool.tile([128, NK], F32, tag="S")
        for kt in range(NKT):
            ps_s = ps_score.tile([128, KT], F32, tag="score")
            nc.tensor.matmul(ps_s[:], lhsT=lhsT,
                             rhs=keys_aug[:, bass.ts(kt, KT)],
                             start=True, stop=True)
            nc.scalar.copy(S[:, bass.ts(kt, KT)], ps_s[:])

        # L1: top-8 values + chunk-local indices per chunk
        Ct = tk.tile([128, CAND], F32, tag="C")
        I1 = tk.tile([128, CAND], U16, tag="I1")
        for c in range(NCH):
            nc.vector.max(out=Ct[:, bass.ts(c, 8)], in_=S[:, bass.ts(c, CH)])
        for c in range(NCH):
            nc.vector.max_index(out=I1[:, bass.ts(c, 8)],
                                in_max=Ct[:, bass.ts(c, 8)],
                                in_values=S[:, bass.ts(c, CH)])
        I1g = tk.tile([128, CAND], U16, tag="I1g")
        nc.vector.tensor_tensor(out=I1g[:], in0=I1[:], in1=cbase[:],
                                op=mybir.AluOpType.add)

        # L2: top-16 values + candidate positions
        T1 = tk.tile([128, 8], F32, tag="T1")
        T2 = tk.tile([128, 8], F32, tag="T2")
        Cmr = tk.tile([128, CAND], F32, tag="Cmr")
        P12 = tk.tile([128, K], U16, tag="P12")
        nc.vector.max(out=T1[:], in_=Ct[:])
        nc.vector.match_replace(out=Cmr[:], in_to_replace=T1[:],
                                in_values=Ct[:], imm_value=-1e30)
        nc.vector.max(out=T2[:], in_=Cmr[:])
        nc.vector.max_index(out=P12[:, 0:8], in_max=T1[:], in_values=Ct[:])
        nc.vector.max_index(out=P12[:, 8:16], in_max=T2[:], in_values=Cmr[:])

        # map candidate positions -> global indices (two local scatters)
        dst1 = tk.tile([128, CAND], I16, tag="dst1")
        nc.gpsimd.local_scatter(dst1[:], iota16[:], P12[:].bitcast(I16),
                                channels=128, num_elems=CAND, num_idxs=K)
        m01 = tk.tile([128, CAND], I16, tag="m01")
        nc.vector.tensor_scalar(out=m01[:], in0=dst1[:], scalar1=0,
                                scalar2=None, op0=mybir.AluOpType.is_gt)
        Z = tk.tile([128, CAND], I16, tag="Z")
        nc.vector.tensor_tensor(out=Z[:], in0=m01[:], in1=I1g[:].bitcast(I16),
                                op=mybir.AluOpType.mult)
        pm1 = tk.tile([128, CAND], I16, tag="pm1")
        nc.vector.tensor_scalar(out=pm1[:], in0=dst1[:], scalar1=1,
                                scalar2=None, op0=mybir.AluOpType.subtract)
        idx16 = tk.tile([128, K], I16, tag="idx16")
        nc.gpsimd.local_scatter(idx16[:], Z[:], pm1[:],
                                channels=128, num_elems=K, num_idxs=CAND)
        idx32 = tk.tile([128, K], U32, tag="idx32")
        nc.vector.tensor_copy(out=idx32[:], in_=idx16[:].bitcast(U16))

        if cfg.get("no_gather"):
            nc.scalar.copy(outm_sb[0:C, bass.ts(qt, 128)],
                           Ct[0:C, :].to_broadcast([C, 128]) if False else Ct[0:C, :])
            continue

        if "dbg_idx" in outs:
            nc.sync.dma_start(out=outs["dbg_idx"][:, bass.ts(qt, K)],
                              in_=idx32[:])
        if qt == 0:
            if "dbg_S" in outs:
                nc.sync.dma_start(out=outs["dbg_S"], in_=S[:])
            if "dbg_C" in outs:
                nc.sync.dma_start(out=outs["dbg_C"], in_=Ct[:])
            if "dbg_I1g" in outs:
                dbgi = tk.tile([128, CAND], U32, tag="dbgi")
                nc.vector.tensor_copy(out=dbgi[:], in_=I1g[:])
                nc.sync.dma_start(out=outs["dbg_I1g"], in_=dbgi[:])
            if "dbg_P" in outs:
                dbgp = tk.tile([128, K], U32, tag="dbgp")
                nc.vector.tensor_copy(out=dbgp[:], in_=P12[:])
                nc.sync.dma_start(out=outs["dbg_P"], in_=dbgp[:])

        # gather neighbor features: G[q, k, :] = v^T[idx[q,k], :]
        # (one indirect DMA per k: HW consumes one offset per partition)
        G = gpool.tile([128, K * C], F32, tag="G")
        for k in range(K):
            nc.gpsimd.indirect_dma_start(
                out=G[:, bass.ts(k, C)], out_offset=None,
                in_=vt_dram,
                in_offset=bass.IndirectOffsetOnAxis(ap=idx32[:, k:k + 1],
                                                    axis=0),
            )

        G_tiles[qt] = G

        if qt >= 2:
            tail(qt - 2)


    if not cfg.get("no_gather"):
        tail(NQT - 2)
        tail(NQT - 1)

    # ---- epilogue: stats psum -> sbuf -> dram; out_m -> dram ----
    s_sb = big.tile([128, KT], F32, tag="s_sb")
    nc.vector.memset(s_sb[:], 0.0)
    H = K * C // 2
    if stats_ps is not None:
        for p in (0, 32, 64, 96):
            nc.scalar.copy(s_sb[p:p + 1, 0:H], stats_ps[p:p + 1, 0:H])
    nc.sync.dma_start(out=out_s, in_=s_sb[:])
    nc.sync.dma_start(out=out_m, in_=outm_sb[:])
    ctx.close()


def build_program(cfg, num_cores=8, reps=1):
    nc = bacc.Bacc("TRN2", target_bir_lowering=False, debug=False,
                   enable_asserts=False, num_devices=num_cores)
    C, NK, NQ = cfg["C"], cfg["NK"], cfg["NQ"]
    ins = {
        "f": nc.dram_tensor("f", [C, NK], F32, kind="ExternalInput").ap(),
        "fq": nc.dram_tensor("fq", [C, NQ], F32, kind="ExternalInput").ap(),
        "w2t": nc.dram_tensor("w2t", [C, C], F32, kind="ExternalInput").ap(),
        "wat": nc.dram_tensor("wat", [C, C], F32, kind="ExternalInput").ap(),
    }
    outs = {
        "out_m": nc.dram_tensor("out_m", [C, NQ], F32,
                                kind="ExternalOutput").ap(),
        "out_s": nc.dram_tensor("out_s", [128, cfg["KT"]], F32,
                                kind="ExternalOutput").ap(),
    }
    with tile.TileContext(nc) as tc:
        for r in range(reps):
            emit(tc, ins, outs, cfg, uniq=f"_r{r}")
    nc.compile()
    return nc


_PROGRAM_CACHE = {}


def get_program(num_cores=8):
    key = num_cores
    if key not in _PROGRAM_CACHE:
        _PROGRAM_CACHE[key] = build_program(default_cfg(), num_cores)
    return _PROGRAM_CACHE[key]


def host_epilogue(m_full, s1, s2, gamma, beta, count):
    """Apply BatchNorm affine + LeakyReLU on the max-reduced tensor."""
    mean = s1 / count
    var = s2 / count - mean * mean
    a = gamma.astype(np.float64) / np.sqrt(var + BN_EPS)
    b = beta.astype(np.float64) - a * mean
    y = a[None, :, None] * m_full.astype(np.float64) + b[None, :, None]
    y = np.where(y >= 0, y, LRELU_SLOPE * y)
    return y.astype(np.float32)


def kernel(x, W, gamma, beta):
    """Full (unsharded) inputs -> full output. See module docstring."""
    from concourse import bass_utils

    x = np.asarray(x)
    W = np.asarray(W)
    gamma = np.asarray(gamma)
    beta = np.asarray(beta)

    B, C, N, _ = x.shape
    K = 16
    assert (B, C, N) == (2, 64, 8192), "kernel hardcoded for this problem size"

    f = np.ascontiguousarray(x[:, :, :, 0])          # (2, 64, 8192)
    W1, W2 = W[:, :C], W[:, C:]
    w2t = np.ascontiguousarray(W2.T)                  # (c, o)
    wat = np.ascontiguousarray((W1 - W2).T)           # (c, o)

    cfg = default_cfg()
    NQ = cfg["NQ"]
    n_cores = 8
    per_batch = N // NQ                               # 4 query blocks per batch

    in_maps = []
    for c in range(n_cores):
        b, qb = c // per_batch, c % per_batch
        in_maps.append({
            "f": np.ascontiguousarray(f[b]),
            "fq": np.ascontiguousarray(f[b][:, qb * NQ:(qb + 1) * NQ]),
            "w2t": w2t,
            "wat": wat,
        })

    nc = get_program(n_cores)
    res = bass_utils.run_bass_kernel_spmd(nc, in_maps, list(range(n_cores)))
    results = res.results

    m_full = np.empty((B, C, N), np.float32)
    s1 = np.zeros(C, np.float64)
    s2 = np.zeros(C, np.float64)
    for c in range(n_cores):
        b, qb = c // per_batch, c % per_batch
        m_full[b, :, qb * NQ:(qb + 1) * NQ] = results[c]["out_m"]
        st = results[c]["out_s"].astype(np.float64)
        H = K * C // 2
        s1 += (st[0, :H].reshape(K // 2, C) + st[32, :H].reshape(K // 2, C)).sum(0)
        s2 += (st[64, :H].reshape(K // 2, C) + st[96, :H].reshape(K // 2, C)).sum(0)

    count = float(B) * N * K
    return host_epilogue(m_full, s1, s2, gamma, beta, count)


if __name__ == "__main__":
    sys.path.insert(0, os.path.dirname(os.path.abspath(__file__)))
    import reference

    inputs = {k: np.asarray(v) for k, v in reference.setup_inputs().items()}
    out = kernel(**inputs)
    exp = np.asarray(reference.reference(**inputs))
    err = np.abs(out - exp)
    rel = np.linalg.norm(out - exp) / np.linalg.norm(exp)
    print("max abs err:", err.max(), "rel l2 err:", rel)



# revision 35
# speedup vs baseline: 1.1442x; 1.1442x over previous
"""Trainium2 Bass kernel for nn_DynConv2d (DGCNN EdgeConv layer).

Reference computation (B=2, C=64, N=8192, K=16, C_out=64):
  f = x[:,:,:,0]
  nn_idx = top-16 nearest neighbors by squared L2 over point features
  feat = concat([x_i, x_j - x_i])          # (B, 2C, N, K)
  y = W @ feat                             # 1x1 conv
  y = BatchNorm2d(y)  (training stats over (B,N,K))
  y = LeakyReLU(0.2)(y)
  out = max over K                         # (B, C_out, N)

Algebraic restructuring:
  * W @ [x_i; x_j - x_i] = u[:,i] + v[:,j] with u = (W1-W2)@f, v = W2@f.
  * BN+LeakyReLU is per-channel monotone, so max over K commutes; the kernel
    returns max_k(u+v_j) plus the BN batch stats (sum / sum-sq); the final
    affine + lrelu runs on host.
  * KNN score s = q.m - |m|^2/2 via a 128-contraction fp16 matmul: rows 0:64
    are features, rows 64:128 of keys hold -f^2/2 (q side holds ones), so no
    separate |m|^2 row materialization is needed.

Top-16 selection (per query row of 8192 fp32 PSUM scores):
  * Scalar engine evacuates PSUM -> SBUF converting to fp16 *into the high
    u16 lane* of a packed fp32 word whose low u16 lane is the key index
    (preloaded from a host template).  Numeric fp32 ordering of the packed
    word == lexicographic (fp16 score, index) ordering, so a single DVE
    Max top-8 per 1024-chunk yields values *and* indices in one pass - no
    MaxIndex, no scatter.
  * A per-query bias (-(q.mbar - c0 + 3.9|q|)) is added during evacuation to
    center the interesting (top-16) scores near zero, which shrinks the fp16
    rounding error where it matters.  Any per-query constant preserves the
    within-row order, so this never breaks correctness.
  * L2: Max + MatchReplace + Max over the 64 chunk-candidates -> top-16
    packed words; low lanes are the global key indices.
  * Indices are relayed out to the dma_gather wrap layout (16 partitions,
    idx[n%16, n//16] = nn-index of slot n = k*128+q) with 16 tiny 16x16 PE
    transposes, then one InstDMAGatherAnt fetches all 2048 neighbor rows of
    v^T per query tile.

Sharding: 8 cores; core c handles batch c//4, query block c%4 (2048 queries),
against all 8192 keys of its batch.
"""

import os
import sys

import numpy as np

sys.path.insert(0, "/opt/trn_rl_repo")

import concourse.bacc as bacc
import concourse.bass as bass
import concourse.mybir as mybir
import concourse.tile as tile
from concourse.masks import make_identity

F32 = mybir.dt.float32
F16 = mybir.dt.float16
U16 = mybir.dt.uint16
I16 = mybir.dt.int16

BN_EPS = 1e-5
LRELU_SLOPE = 0.2
BIAS_KAPPA = 3.9


def default_cfg():
    return dict(C=64, NK=8192, NQ=2048, K=16, CH=1024)


def emit(tc, ins, outs, cfg, uniq=""):
    """Per-core program.

    ins:  f (64, NK) f16, fq (64, NQ) f16 (this core's query slice),
          w2t (C, C) f16, wat (C, C) f16,
          mb (128, 1) f16  [rows 0:64 = -mean_keys(f), rows 64:128 = c0/64],
          pkt (128, NK) f32 [u32 words: low u16 = column index, high = 0]
    outs: out_o (NQ, C) f16   max-over-K of u+v (pre-BN), query-major
          out_s (128, 512) f32  rows {0,32,64,96} = psum stats
    """
    nc = tc.nc
    C = cfg["C"]          # 64
    NK = cfg["NK"]        # 8192
    NQ = cfg["NQ"]        # 2048
    K = cfg["K"]          # 16
    CH = cfg["CH"]        # 1024  L1 top-8 chunk
    NQT = NQ // 128       # 16 query tiles
    NCH = NK // CH        # 8 chunks
    CAND = 8 * NCH        # 64 candidates
    H = K * C // 2        # 512 (stats half-width)

    f, fq, w2t, wat, mb, pkt = (ins["f"], ins["fq"], ins["w2t"], ins["wat"],
                                ins["mb"], ins["pkt"])
    out_o, out_s = outs["out_o"], outs["out_s"]

    from contextlib import ExitStack
    ctx = ExitStack()
    dram_pool = ctx.enter_context(tc.tile_pool(name="dram" + uniq, bufs=1,
                                               space="DRAM"))
    vt_dram = dram_pool.tile([NK, C], F32, tag="vt", name="vt_t" + uniq)[:]

    cpool = ctx.enter_context(tc.tile_pool(name="consts" + uniq, bufs=1))
    big = ctx.enter_context(tc.tile_pool(name="big" + uniq, bufs=1))
    tk = ctx.enter_context(tc.tile_pool(name="topk" + uniq, bufs=2))
    vpool = ctx.enter_context(tc.tile_pool(name="vstage" + uniq, bufs=8))
    gpool = ctx.enter_context(tc.tile_pool(name="gather" + uniq, bufs=3))
    tpool = ctx.enter_context(tc.tile_pool(name="tail" + uniq, bufs=2))
    ps_score = ctx.enter_context(tc.tile_pool(name="ps_score" + uniq, bufs=2,
                                              space="PSUM"))
    ps_stat = ctx.enter_context(tc.tile_pool(name="ps_stat" + uniq, bufs=1,
                                             space="PSUM"))
    ps_misc = ctx.enter_context(tc.tile_pool(name="ps_misc" + uniq, bufs=3,
                                             space="PSUM"))

    # ---- constants / inputs ----
    identity = cpool.tile([128, 128], F32, tag="ident")
    make_identity(nc, identity[:])
    ones64 = cpool.tile([64, 1], F16, tag="ones64")
    nc.vector.memset(ones64[:], 1.0)
    ones128 = cpool.tile([128, 1], F16, tag="ones128")
    nc.vector.memset(ones128[:], 1.0)
    # w2t lives at base partition 64 to pair with keys_aug[64:128] in matmuls
    w2t_sb = cpool.tile([128, C], F16, tag="w2t")
    nc.sync.dma_start(out=w2t_sb[64:128, :], in_=w2t)
    wat_sb = cpool.tile([C, C], F16, tag="wat")
    nc.sync.dma_start(out=wat_sb[:], in_=wat)
    mb_sb = cpool.tile([128, 1], F16, tag="mb")
    nc.sync.dma_start(out=mb_sb[:], in_=mb)

    # keys_aug: rows 0:64 = f, rows 64:128 = -f^2/2 (built in place from a
    # second copy of f so every engine op stays partition-aligned).
    # DMAs are spread across engine queues so the prologue isn't serialized
    # on SP.
    # Prologue is scheduled at quarter granularity so the first score matmul
    # (which needs keys_aug rows 0:128 incl. -f^2/2) can start ~4us in:
    #   SP queue:  w/mb, q_aug, keys1 quarters, pkt0 chunk-pieces, keys1 rest
    #   Act queue: keys2 quarters (feeds vt matmuls + f^2)
    #   Pool:      f^2 quarters (after the vt matmuls that read raw f), pkt1
    #   DVE:       q2 square, vt/ut psum->sbuf copies
    keys_aug = big.tile([128, NK], F16, tag="keys_aug")
    q_aug = big.tile([128, NQ], F16, tag="q_aug")
    nc.sync.dma_start(out=q_aug[0:64, :], in_=fq)
    # rows 64:128 hold -0.5 so the score matmul contracts -0.5 * f^2 rows
    nc.gpsimd.memset(q_aug[64:128, :], -0.5)
    NQR = NK // 4
    for r in range(4):
        nc.scalar.dma_start(out=keys_aug[64:128, bass.ts(r, NQR)],
                            in_=f[:, bass.ts(r, NQR)])

    # fq^2 for the per-query |q| bias estimate (DVE, 2-byte fast path)
    q2 = big.tile([64, NQ], F16, tag="q2")
    nc.vector.tensor_tensor(out=q2[:], in0=q_aug[0:64, :], in1=q_aug[0:64, :],
                            op=mybir.AluOpType.mult)

    # Single packed score buffer (low u16 lanes = key index template).  The
    # evac(t+1, c) -> Max(t, c) WAR is 8 chunk-slots stale, so one buffer
    # pipelines with no stalls.  keys1 quarters and template chunks
    # interleave on SP so chunk c's template lands just before Max(0, c).
    packed = big.tile([128, NK], F32, tag="packed")
    for r in range(4):
        nc.sync.dma_start(out=keys_aug[0:64, bass.ts(r, NQR)],
                          in_=f[:, bass.ts(r, NQR)])
        if r < 2:
            for c in (2 * r, 2 * r + 1):
                nc.sync.dma_start(out=packed[:, c * CH:(c + 1) * CH],
                                  in_=pkt[:, c * CH:(c + 1) * CH])
    # gather index tile (rows 16:128 must stay 0 for the executor's bounds
    # check; only rows 0:16 are consumed)
    gtidx = big.tile([128, 128], I16, tag="gtidx")
    nc.gpsimd.memset(gtidx[:], 0)

    # ---- v^T matmuls (read raw f in rows 64:128) + quartered -f^2/2 ----
    # Emission order per quarter: vt matmuls reading that quarter first, then
    # the in-place squaring of the same columns (readers before writer).
    vt_view = vt_dram.rearrange("(t c p) ch -> t p c ch", t=8, c=8)
    for t in range(8):
        ps_vt = ps_misc.tile([128, 512], F32, tag="misc")
        for c in range(8):
            nc.tensor.matmul(ps_vt[:, c * C:(c + 1) * C],
                             lhsT=keys_aug[64:128, bass.ts(8 * t + c, 128)],
                             rhs=w2t_sb[64:128, :], start=True, stop=True)
        vt_sb = vpool.tile([128, 512], F32, tag="vt_sb")
        nc.vector.tensor_copy(out=vt_sb[:], in_=ps_vt[:])
        nc.sync.dma_start(out=vt_view[t],
                          in_=vt_sb[:].rearrange("p (c ch) -> p c ch", c=8))
        if t % 2 == 1:
            r = t // 2
            nc.gpsimd.tensor_tensor(
                out=keys_aug[64:128, bass.ts(r, NQR)],
                in0=keys_aug[64:128, bass.ts(r, NQR)],
                in1=keys_aug[64:128, bass.ts(r, NQR)],
                op=mybir.AluOpType.mult)
    # late template chunks ride the Pool DMA queue (deadline: Max(0, c))
    for c in range(4, NCH):
        nc.gpsimd.dma_start(out=packed[:, c * CH:(c + 1) * CH],
                            in_=pkt[:, c * CH:(c + 1) * CH])

    # per-query bias for all tiles: bias_all[:, t] = -(q.mbar - c0 + kappa*|q|)
    ps_ball = ps_misc.tile([128, 512], F32, tag="misc", name="psball" + uniq)
    for qt in range(NQT):
        nc.tensor.matmul(ps_ball[:, qt:qt + 1], lhsT=q_aug[:, bass.ts(qt, 128)],
                         rhs=mb_sb[:], start=True, stop=True)
        nc.tensor.matmul(ps_ball[:, NQT + qt:NQT + qt + 1],
                         lhsT=q2[:, bass.ts(qt, 128)], rhs=ones64[:],
                         start=True, stop=True)
    qn_all = big.tile([128, NQT], F32, tag="qn_all")
    nc.scalar.activation(qn_all[:], ps_ball[:, NQT:2 * NQT],
                         mybir.ActivationFunctionType.Sqrt, scale=1.0)
    bias_all = big.tile([128, NQT], F32, tag="bias_all")
    nc.vector.scalar_tensor_tensor(
        out=bias_all[:], in0=qn_all[:], scalar=-BIAS_KAPPA,
        in1=ps_ball[:, 0:NQT],
        op0=mybir.AluOpType.mult, op1=mybir.AluOpType.add)

    ut_sb = big.tile([128, NQT * C], F32, tag="ut")
    for h in range(2):
        ps_ut = ps_misc.tile([128, 512], F32, tag="misc")
        for j in range(8):
            nc.tensor.matmul(ps_ut[:, j * C:(j + 1) * C],
                             lhsT=q_aug[0:64, bass.ts(8 * h + j, 128)],
                             rhs=wat_sb[:], start=True, stop=True)
        nc.vector.tensor_copy(out=ut_sb[:, h * 512:(h + 1) * 512], in_=ps_ut[:])

    stats_ps = ps_stat.tile([128, 512], F32, tag="stat")

    # ---- per-tile tail: A = G + u, stats, max over K, out DMA ----
    def tail(tq):
        G = G_tiles.pop(tq)
        ut_qt = ut_sb[:, bass.ts(tq, C)]
        A = tpool.tile([128, K * C], F16, tag="A")
        nc.gpsimd.tensor_tensor(
            out=A[:].rearrange("p (k o) -> p k o", k=K),
            in0=G[:].rearrange("p (k o) -> p k o", k=K),
            in1=ut_qt.unsqueeze(1).broadcast_to([128, K, C]),
            op=mybir.AluOpType.add)
        A2 = tpool.tile([128, K * C], F16, tag="A2")
        nc.gpsimd.tensor_tensor(out=A2[:], in0=A[:], in1=A[:],
                                op=mybir.AluOpType.mult)
        st, sp = (tq == 0), (tq == NQT - 1)
        nc.tensor.matmul(stats_ps[0:1, 0:H], lhsT=ones128[:], rhs=A[:, 0:H],
                         start=st, stop=sp, tile_position=(0, 0))
        nc.tensor.matmul(stats_ps[32:33, 0:H], lhsT=ones128[:], rhs=A[:, H:],
                         start=st, stop=sp, tile_position=(0, 32))
        nc.tensor.matmul(stats_ps[64:65, 0:H], lhsT=ones128[:], rhs=A2[:, 0:H],
                         start=st, stop=sp, tile_position=(0, 64))
        nc.tensor.matmul(stats_ps[96:97, 0:H], lhsT=ones128[:], rhs=A2[:, H:],
                         start=st, stop=sp, tile_position=(0, 96))
        # max over K: fold tree (8,4,2,1) on DVE fp16 (2x path; Pool has no
        # ISA max, and contiguous halves keep the 2-byte fast mode)
        M1 = tpool.tile([128, 8 * C], F16, tag="M1")
        nc.vector.tensor_tensor(out=M1[:], in0=A[:, 0:8 * C], in1=A[:, 8 * C:],
                                op=mybir.AluOpType.max)
        M2 = tpool.tile([128, 4 * C], F16, tag="M2")
        nc.vector.tensor_tensor(out=M2[:], in0=M1[:, 0:4 * C], in1=M1[:, 4 * C:],
                                op=mybir.AluOpType.max)
        M3 = tpool.tile([128, 2 * C], F16, tag="M3")
        nc.vector.tensor_tensor(out=M3[:], in0=M2[:, 0:2 * C], in1=M2[:, 2 * C:],
                                op=mybir.AluOpType.max)
        O = tpool.tile([128, C], F16, tag="O")
        nc.vector.tensor_tensor(out=O[:], in0=M3[:, 0:C], in1=M3[:, C:],
                                op=mybir.AluOpType.max)
        nc.sync.dma_start(out=out_o[bass.ts(tq, 128), :], in_=O[:])

    # ---- main loop over query tiles (software pipelined) ----
    # stage A(t): scores -> packed -> L1/L2 top-16 -> idxf
    # stage B(t): idx relayout (PE transposes) -> gtidx -> dma_gather
    # stage C(t): tail (A = G+u, stats, max over K, out DMA)
    # Emission A(t), B(t-1), C(t-2) keeps each in-order engine queue from
    # stalling tile t's independent work behind tile t-1's cross-engine chain.
    G_tiles = {}
    idx_tiles = {}

    def stageA(qt):
        qcols = bass.ts(qt, 128)
        pk = packed
        pk_hi = pk[:].bitcast(F16).rearrange("p (n two) -> p n two", two=2)

        # scores -> packed fp16-high-lane (+bias) -> L1 top-8 per chunk
        Ct = tk.tile([128, CAND], F32, tag="C")
        for c in range(NCH):
            ps_s = ps_score.tile([128, CH], F32, tag="score",
                                 name=f"pss{uniq}_{qt}_{c}")
            nc.tensor.matmul(ps_s[:, 0:512], lhsT=q_aug[:, qcols],
                             rhs=keys_aug[:, c * CH:c * CH + 512],
                             start=True, stop=True)
            nc.tensor.matmul(ps_s[:, 512:1024], lhsT=q_aug[:, qcols],
                             rhs=keys_aug[:, c * CH + 512:(c + 1) * CH],
                             start=True, stop=True)
            nc.scalar.activation(pk_hi[:, c * CH:(c + 1) * CH, 1:2], ps_s[:],
                                 mybir.ActivationFunctionType.Identity,
                                 bias=bias_all[:, qt:qt + 1], scale=1.0)
            nc.vector.max(out=Ct[:, bass.ts(c, 8)],
                          in_=pk[:, c * CH:(c + 1) * CH])

        # L2: top-16 packed words
        T12 = tk.tile([128, K], F32, tag="T12")
        Cmr = tk.tile([128, CAND], F32, tag="Cmr")
        nc.vector.max(out=T12[:, 0:8], in_=Ct[:])
        nc.vector.match_replace(out=Cmr[:], in_to_replace=T12[:, 0:8],
                                in_values=Ct[:], imm_value=-1e30)
        nc.vector.max(out=T12[:, 8:16], in_=Cmr[:])
        idxf = tk.tile([128, K], F32, tag="idxf")
        nc.vector.tensor_copy(
            out=idxf[:].unsqueeze(2),
            in_=T12[:].bitcast(U16).rearrange("p (n two) -> p n two",
                                              two=2)[:, :, 0:1])
        idx_tiles[qt] = idxf

    def stageB(qt):
        idxf = idx_tiles.pop(qt)
        # relayout idx [128q, 16k] -> wrap layout [16, 128]:
        # X[k, q] = idx[q, k] (one full transpose), then per-16-block
        # transposes ps_y[b, 16a+k] = X[k, 16a+b] (all base-partition 0).
        # The block transposes take a free-duplicated input so the output
        # lands twice (partitions 0:16 and 16:32) - the gather's tx Q7 core
        # reads the index rows from partitions 16:32.
        ps_x = ps_misc.tile([128, 512], F32, tag="misc", name=f"psx{uniq}_{qt}")
        nc.tensor.transpose(ps_x[0:16, 0:128], idxf[:], identity[:])
        Xsb = tk.tile([16, 128], F32, tag="Xsb")
        nc.scalar.copy(Xsb[:], ps_x[0:16, 0:128])
        ps_y = ps_misc.tile([128, 512], F32, tag="misc", name=f"psy{uniq}_{qt}")
        for a in range(8):
            nc.tensor.transpose(ps_y[0:16, 16 * a:16 * (a + 1)],
                                Xsb[:, 16 * a:16 * (a + 1)],
                                identity[0:16, 0:16])
        gt = gtidx
        # gtidx[b, 8k+a] = ps_y[b, 16a+k]; the gather's tx Q7 core reads the
        # index rows from partitions 16:32, so DMA-replicate them there.
        nc.scalar.activation(
            gt[0:16, :],
            ps_y[0:16, 0:128].rearrange("p (a k) -> p k a", a=8),
            mybir.ActivationFunctionType.Copy)
        nc.sync.dma_start(out=gt[16:32, :], in_=gt[0:16, :])

        # gather neighbor features G[q, k, :] = v^T[nn[q, k], :]
        # (4 ops of 512 idxs - the Q7 idx scratch caps num_idxs at 512)
        G = gpool.tile([128, K * C], F32, tag="G")
        Gv = G[:].rearrange("p (k o) -> p k o", k=K)
        for g in range(4):
            nc.gpsimd.dma_gather(
                out_ap=Gv[:, 4 * g:4 * (g + 1), :],
                in_ap=vt_dram,
                idxs_ap=gt[:, 32 * g:32 * (g + 1)],
                num_idxs=512,
                num_idxs_reg=512,
                elem_size=C,
            )
        G_tiles[qt] = G

    for qt in range(NQT):
        stageA(qt)
        if qt >= 1:
            stageB(qt - 1)
        if qt >= 2:
            tail(qt - 2)
    stageB(NQT - 1)
    tail(NQT - 2)
    tail(NQT - 1)

    # ---- epilogue: stats psum -> sbuf -> dram ----
    s_sb = big.tile([128, 512], F32, tag="s_sb")
    nc.vector.memset(s_sb[:], 0.0)
    for p in (0, 32, 64, 96):
        nc.scalar.copy(s_sb[p:p + 1, :], stats_ps[p:p + 1, :])
    nc.sync.dma_start(out=out_s, in_=s_sb[:])
    ctx.close()


def build_program(cfg, num_cores=8, reps=1):
    nc = bacc.Bacc("TRN2", target_bir_lowering=False, debug=False,
                   enable_asserts=False, num_devices=num_cores)
    C, NK, NQ = cfg["C"], cfg["NK"], cfg["NQ"]
    ins = {
        "f": nc.dram_tensor("f", [C, NK], F16, kind="ExternalInput").ap(),
        "fq": nc.dram_tensor("fq", [C, NQ], F16, kind="ExternalInput").ap(),
        "w2t": nc.dram_tensor("w2t", [C, C], F16, kind="ExternalInput").ap(),
        "wat": nc.dram_tensor("wat", [C, C], F16, kind="ExternalInput").ap(),
        "mb": nc.dram_tensor("mb", [128, 1], F16, kind="ExternalInput").ap(),
        "pkt": nc.dram_tensor("pkt", [128, NK], F32, kind="ExternalInput").ap(),
    }
    outs = {
        "out_o": nc.dram_tensor("out_o", [NQ, C], F16,
                                kind="ExternalOutput").ap(),
        "out_s": nc.dram_tensor("out_s", [128, 512], F32,
                                kind="ExternalOutput").ap(),
    }
    with tile.TileContext(nc) as tc:
        for r in range(reps):
            emit(tc, ins, outs, cfg, uniq=f"_r{r}")
    nc.compile()
    return nc


_PROGRAM_CACHE = {}


def get_program(num_cores=8):
    key = num_cores
    if key not in _PROGRAM_CACHE:
        _PROGRAM_CACHE[key] = build_program(default_cfg(), num_cores)
    return _PROGRAM_CACHE[key]


def make_in_maps(x, W, n_cores=8):
    """Build the per-core input dicts from the full inputs."""
    cfg = default_cfg()
    C, NQ = cfg["C"], cfg["NQ"]
    B, _, N, _ = x.shape
    per_batch = N // NQ
    f16 = np.ascontiguousarray(x[:, :, :, 0]).astype(np.float16)  # (B, C, N)
    W16 = W.astype(np.float16)
    W1, W2 = W16[:, :C], W16[:, C:]
    w2t = np.ascontiguousarray(W2.T)
    wat = np.ascontiguousarray((W1 - W2).T)
    pkt = np.broadcast_to(
        np.arange(N, dtype=np.uint32)[None, :], (128, N)).copy().view(np.float32)
    mbs = []
    for b in range(B):
        fb = f16[b].astype(np.float32)
        mbar = fb.mean(axis=1)                      # (C,)
        c0 = float(0.5 * (fb * fb).sum(axis=0).mean())
        mbv = np.zeros((128, 1), np.float16)
        mbv[0:C, 0] = (-mbar).astype(np.float16)
        # contracted against the -0.5 rows of q_aug: 64 * (-0.5) * (-c0/32) = c0
        mbv[C:128, 0] = np.float16(-c0 / 32.0)
        mbs.append(mbv)
    in_maps = []
    for c in range(n_cores):
        b, qb = c // per_batch, c % per_batch
        in_maps.append({
            "f": np.ascontiguousarray(f16[b]),
            "fq": np.ascontiguousarray(f16[b][:, qb * NQ:(qb + 1) * NQ]),
            "w2t": w2t,
            "wat": wat,
            "mb": mbs[b],
            "pkt": pkt,
        })
    return in_maps


def host_epilogue(m_full, s1, s2, gamma, beta, count):
    mean = s1 / count
    var = s2 / count - mean * mean
    a = gamma.astype(np.float64) / np.sqrt(var + BN_EPS)
    b = beta.astype(np.float64) - a * mean
    y = a[None, :, None] * m_full.astype(np.float64) + b[None, :, None]
    y = np.where(y >= 0, y, LRELU_SLOPE * y)
    return y.astype(np.float32)


def kernel(x, W, gamma, beta):
    """Full (unsharded) inputs -> full output. See module docstring."""
    from concourse import bass_utils

    x = np.asarray(x)
    W = np.asarray(W)
    gamma = np.asarray(gamma)
    beta = np.asarray(beta)

    B, C, N, _ = x.shape
    K = 16
    assert (B, C, N) == (2, 64, 8192), "kernel hardcoded for this problem size"

    cfg = default_cfg()
    NQ = cfg["NQ"]
    n_cores = 8
    per_batch = N // NQ

    in_maps = make_in_maps(x, W, n_cores)
    nc = get_program(n_cores)
    res = bass_utils.run_bass_kernel_spmd(nc, in_maps, list(range(n_cores)))
    results = res.results

    m_full = np.empty((B, C, N), np.float32)
    s1 = np.zeros(C, np.float64)
    s2 = np.zeros(C, np.float64)
    H = K * C // 2
    for c in range(n_cores):
        b, qb = c // per_batch, c % per_batch
        m_full[b, :, qb * NQ:(qb + 1) * NQ] = \
            results[c]["out_o"].astype(np.float32).T
        st = results[c]["out_s"].astype(np.float64)
        s1 += (st[0, :H].reshape(K // 2, C) + st[32, :H].reshape(K // 2, C)).sum(0)
        s2 += (st[64, :H].reshape(K // 2, C) + st[96, :H].reshape(K // 2, C)).sum(0)

    count = float(B) * N * K
    return host_epilogue(m_full, s1, s2, gamma, beta, count)


if __name__ == "__main__":
    sys.path.insert(0, os.path.dirname(os.path.abspath(__file__)))
    import reference

    inputs = {k: np.asarray(v) for k, v in reference.setup_inputs().items()}
    out = kernel(**inputs)
    import jax
    cpu = jax.devices("cpu")[0]
    with jax.default_device(cpu):
        exp = np.asarray(reference.reference(
            **{k: jax.device_put(v, cpu) for k, v in inputs.items()}))
    err = np.abs(out - exp)
    rel = np.linalg.norm(out - exp) / np.linalg.norm(exp)
    print("max abs err:", err.max(), "rel l2 err:", rel)
